# revision 45
# baseline (speedup 1.0000x reference)
"""Fused RNN cell on 8 Trainium2 NeuronCores.

Reference computation (fp32):
    combined   = [x, hidden]                      [B=4096, I+H=4096]
    new_hidden = tanh(combined @ W_ih^T + b_ih)   [B, H=2048]
    output     = new_hidden @ W_ho^T + b_ho       [B, O=2048]
    returns (output, new_hidden)

Strategy: data-parallel over the batch — each of the 8 cores processes 512
batch rows with replicated weights; no collectives. All operand layout
transforms (transposes into PE-friendly [K-partition, free] form) happen on
the host so every device DMA is a fat, fully contiguous transfer:

    c   [128, 32, 512]      cL[ki, ko, b]       = combined[b, ko*128+ki]
    w1  [128, 32, 16, 128]  w1L[ki, ko, hc, h]  = W_ih[hc*128+h, ko*128+ki]
    w2  [128, 16, 16, 128]  w2L[hi, ho, oc, o]  = W_ho[oc*128+o, ho*128+hi]
    b1  [128, 16]           b1L[p, hc]          = b_ih[hc*128+p]

All matmul operands are fp16 (full PE rate, 1 col/cycle; fp32 PSUM
accumulation; rms rel err ~5e-4), which halves HBM traffic vs fp32 —
28 MB in + 4 MB out per core vs the ~166 us PE floor, so the kernel is
cleanly compute-bound. (fp8 DoubleRow was measured at 2x PE rate but
e4m3's 3-bit mantissa gives ~5% rms output error vs the 2e-2 gate, and
hi/lo-split schemes cost >= 2 double-rate matmuls = no gain over fp16,
so 16-bit is the fastest dtype that meets accuracy.)

DMA is split across the two HWDGE rings so neither saturates (~186 GB/s
per-ring packet-rate limit) and the first tiles land ASAP: the sync
ring carries only weight loads (w1, w2); the scalar (ACT) ring carries
c loads, b1, nh stores, and out stores. The whole first mm1 group uses
1-ko w1 slices (256 KB) — while the ring pipeline fills, halving the
per-slice transfer keeps delivery ahead of the PE (measured: zero
matmul stalls); the rest uses 0.5 MB 2-ko slices to halve trigger
overhead (~0.65 us queue time per DMA trigger). Dummy matmuls at t=0
warm the PE clock gate (HAM, ~3.4 us busy to reach 2.4 GHz) during the
initial DMA ramp, and a 1-element tanh preloads the ACT table set
(~2.7 us) behind the first c triggers.

mm1 drains: 8 back-to-back tanh ACTs per PSUM group (bias fused), then
the 8 nh store triggers — triggers never sit between ACT ops, so bank
turnaround stays at copy cadence and the next group never stalls. mm2
drains alternate DVE/ACT copies the same way; out-store triggers ride
the scalar ring after the group's copies. mm2 groups are [8, 6, 2] so
only two small stores (on parallel rings) remain after the final
matmul; the final contraction step stops the scalar-ring chunk one
matmul early so its slower drain path gets a head start, and tiny
keep-warm dump-stores keep both DGE rings active into the tail
(measured ~0.3-0.6 us on the exec tail, A/B over drift-interleaved
batches).

Measured: ~183 us HW exec = ~6.5 us framework prologue + ~4 us DMA
fill/clock ramp + ~166 us back-to-back matmuls (PE roofline, zero
stalls) + ~3 us store tail + ~2 us teardown. Rel err ~4.7e-4.
"""

import numpy as np

import concourse.bass as bass
import concourse.mybir as mybir
import concourse.tile as tile
from concourse import bacc, bass_utils

NCORES = 8
B, I, H, O = 4096, 2048, 2048, 2048
BC = B // NCORES          # 512 batch rows per core
K1 = I + H                # mm1 contraction dim, 4096
KO1 = K1 // 128           # 32 k-chunks for mm1
HC = H // 128             # 16 h-chunks
OC = O // 128             # 16 o-chunks
G = 8                     # h-chunks per PSUM group (8 banks)
P = 128
F32 = mybir.dt.float32
F16 = mybir.dt.float16
AF = mybir.ActivationFunctionType
NWARM = 34                # dummy matmuls covering the DMA ramp (~107ns each)


def _build():
    nc = bacc.Bacc("TRN2", target_bir_lowering=False,
                   enable_partition_id=False)

    c = nc.dram_tensor("c", [P, KO1, BC], F16, kind="ExternalInput")
    w1 = nc.dram_tensor("w1", [P, KO1, HC, P], F16, kind="ExternalInput")
    b1 = nc.dram_tensor("b1", [P, HC], F32, kind="ExternalInput")
    w2 = nc.dram_tensor("w2", [P, HC, OC, P], F16, kind="ExternalInput")
    nhT = nc.dram_tensor("nhT", [H, BC], F16, kind="ExternalOutput")
    outT = nc.dram_tensor("outT", [O, BC], F16, kind="ExternalOutput")
    # 128-byte dump targets for the tail keep-warm stores.
    dump_a = nc.dram_tensor("dump_a", [1, 64], F16, kind="Internal")
    dump_b = nc.dram_tensor("dump_b", [1, 64], F16, kind="Internal")

    with tile.TileContext(nc) as tc:
        with tc.tile_pool(name="cpool", bufs=1) as cpool, \
             tc.tile_pool(name="wpool", bufs=10) as wpool, \
             tc.tile_pool(name="nhpool", bufs=1) as nhpool, \
             tc.tile_pool(name="opool", bufs=8) as opool, \
             tc.tile_pool(name="bpool", bufs=1) as bpool, \
             tc.tile_pool(name="ps", bufs=8, space="PSUM") as ps:

            # Warm-up operand first: the sooner the memset lands, the
            # sooner the PE warm-ups can start spinning up the clock.
            warm_sb = bpool.tile([P, P], mybir.dt.bfloat16)
            nc.vector.memset(warm_sb[:], 0.0)

            b1_sb = bpool.tile([P, HC], F32)

            c_sb = cpool.tile([P, KO1, BC], F16)
            nh_sb = nhpool.tile([P, HC, BC], F16)

            # Scalar-ring head: the first c chunks (needed by the first
            # real matmuls; the very first as its own small transfer so
            # matmuls can start before the rest lands) go out before the
            # ACT tanh-table preload blocks the queue for ~2.7 us.
            nc.scalar.dma_start(c_sb[:, 0:1], c[:, 0:1])
            nc.scalar.dma_start(c_sb[:, 1:2], c[:, 1:2])
            nc.scalar.dma_start(c_sb[:, 2:4], c[:, 2:4])
            nc.scalar.dma_start(c_sb[:, 4:6], c[:, 4:6])
            act_warm = bpool.tile([1, 1], F32)
            nc.scalar.activation(act_warm[:], warm_sb[:1, :1], AF.Tanh)
            for ko0 in range(6, KO1, 2):
                nc.scalar.dma_start(c_sb[:, ko0:ko0 + 2], c[:, ko0:ko0 + 2])
            # b_ih rides the scalar ring behind the c stream (it isn't
            # needed until the first group drains, ~60 us in); keeping it
            # off GpSimd SWDGE avoids the expensive dge_drain at teardown.
            nc.scalar.dma_start(b1_sb[:], b1[:])

            # mm1: nh^T[h, b] = tanh(W_ih @ combined^T + b_ih)
            # Two G-sized PSUM groups ping-pong across the 8 banks.
            for g in range(HC // G):
                psums = [ps.tile([P, BC], F32, tag="ps", name=f"ps{i}")
                         for i in range(G)]
                if g == 0:
                    # PE warm-up: HAM holds the PE at 1.2 GHz until ~3.4 us
                    # of busy time. Dummy matmuls (into the last bank this
                    # group will touch; start=True on the real group clears
                    # it) keep the PE active while the first tiles stream
                    # in, so real matmuls run near 2.4 GHz from the start.
                    for _ in range(NWARM):
                        nc.tensor.matmul(
                            psums[G - 1][:, :P], lhsT=warm_sb[:],
                            rhs=warm_sb[:],
                            start=True, stop=True, skip_group_check=True,
                        )
                    # 256 KB 1-ko slices throughout the first group: the
                    # ring pipeline is still filling, and halving the
                    # per-slice transfer keeps delivery ahead of the PE.
                    slices = [(ko, 1, 0, G) for ko in range(KO1)]
                else:
                    slices = [(ko, 2, 0, G) for ko in range(0, KO1, 2)]
                for ko0, kw, i0, ni in slices:
                    w1_sb = wpool.tile([P, 2, G, P], F16, tag="w")
                    nc.sync.dma_start(
                        w1_sb[:, :kw, :ni],
                        w1[:, ko0:ko0 + kw, g * G + i0:g * G + i0 + ni])
                    for kk in range(kw):
                        for i in range(ni):
                            nc.tensor.matmul(
                                psums[i0 + i][:],
                                lhsT=w1_sb[:, kk, i],
                                rhs=c_sb[:, ko0 + kk],
                                start=(ko0 + kk == 0),
                                stop=(ko0 + kk == KO1 - 1),
                            )
                # Back-to-back tanhs first (bank turnaround at ACT copy
                # cadence), store triggers after.
                for i in range(G):
                    hc = g * G + i
                    nc.scalar.activation(
                        nh_sb[:, hc], psums[i][:], AF.Tanh,
                        bias=b1_sb[:, hc:hc + 1],
                    )
                for i in range(G):
                    hc = g * G + i
                    nc.scalar.dma_start(
                        nhT[hc * P:(hc + 1) * P, :], nh_sb[:, hc])

            # mm2: out^T[o, b] = W_ho @ nh^T (+ b_ho on host)
            # Groups of [8, 6, 2] o-chunks: consecutive groups ping-pong
            # through the 8 PSUM banks, and the final drain after the last
            # matmul is just two chunks on two parallel rings.
            for g0, gsz in ((0, 8), (8, 6), (14, 2)):
                psums = [ps.tile([P, BC], F32, tag="ps", name=f"ps{i}")
                         for i in range(gsz)]
                for ho0 in range(0, HC, 2):
                    w2_sb = wpool.tile([P, 2, G, P], F16, tag="w",
                                       name="w2_sb")[:, :, :gsz]
                    nc.sync.dma_start(
                        w2_sb[:], w2[:, ho0:ho0 + 2, g0:g0 + gsz])
                    if gsz == 2 and ho0 == HC - 2:
                        # Keep-warm: tiny dump-stores on both DGE rings,
                        # dependent on the final w2 slice load, so the
                        # engines are freshly active when the two real
                        # tail store triggers land moments later.
                        nc.sync.dma_start(dump_a[:], w2_sb[:1, 0, 0, :64])
                        nc.scalar.dma_start(dump_b[:], w2_sb[:1, 0, 1, :64])
                    for kk in range(2):
                        order = range(gsz)
                        if gsz == 2 and ho0 == HC - 2 and kk == 1:
                            # Final step: stop chunk 15 (the slower
                            # ACT+scalar drain path) one matmul BEFORE
                            # chunk 14, so its copy starts ~216 ns early.
                            order = (1, 0)
                        for i in order:
                            nc.tensor.matmul(
                                psums[i][:],
                                lhsT=w2_sb[:, kk, i],
                                rhs=nh_sb[:, ho0 + kk],
                                start=(ho0 + kk == 0),
                                stop=(ho0 + kk == HC - 1),
                            )
                # Alternate DVE/ACT copies back-to-back, then the store
                # triggers (scalar ring; the sync ring takes the chunk that
                # stops last so the final drain runs on parallel rings).
                o_sbs = []
                for i in range(gsz):
                    o_sb = opool.tile([P, BC], F16, tag="osb")
                    if i % 2:
                        nc.scalar.activation(o_sb[:], psums[i][:], AF.Copy)
                    else:
                        nc.vector.tensor_copy(o_sb[:], psums[i][:])
                    o_sbs.append(o_sb)
                for i in range(gsz):
                    oc = g0 + i
                    eng = nc.sync if (gsz == 2 and i == 0) else nc.scalar
                    eng.dma_start(outT[oc * P:(oc + 1) * P, :], o_sbs[i][:])

    nc.compile()
    return nc


def _shard_inputs(x, hidden, W_ih, b_ih, W_ho, b_ho):
    combined = np.concatenate([x, hidden], axis=1)  # [B, K1]
    w1L = np.ascontiguousarray(
        W_ih.reshape(HC, P, KO1, P).transpose(3, 2, 0, 1).astype(np.float16)
    )  # [ki, ko, hc, h]
    w2L = np.ascontiguousarray(
        W_ho.reshape(OC, P, HC, P).transpose(3, 2, 0, 1).astype(np.float16)
    )  # [hi, ho, oc, o]
    b1L = np.ascontiguousarray(b_ih.reshape(HC, P).T)
    in_maps = []
    for cix in range(NCORES):
        cc = combined[cix * BC:(cix + 1) * BC]  # [BC, K1]
        cL = np.ascontiguousarray(
            cc.reshape(BC, KO1, P).transpose(2, 1, 0).astype(np.float16))
        in_maps.append(
            {"c": cL, "w1": w1L, "b1": b1L, "w2": w2L}
        )
    return in_maps


def _run(in_maps, **kwargs):
    nc = _build()
    return bass_utils.run_bass_kernel_spmd(
        nc, in_maps, core_ids=list(range(NCORES)), **kwargs
    )


def kernel(x, hidden, W_ih, b_ih, W_ho, b_ho):
    x = np.asarray(x, dtype=np.float32)
    hidden = np.asarray(hidden, dtype=np.float32)
    W_ih = np.asarray(W_ih, dtype=np.float32)
    b_ih = np.asarray(b_ih, dtype=np.float32)
    W_ho = np.asarray(W_ho, dtype=np.float32)
    b_ho = np.asarray(b_ho, dtype=np.float32)

    in_maps = _shard_inputs(x, hidden, W_ih, b_ih, W_ho, b_ho)
    res = _run(in_maps)
    output = np.concatenate(
        [r["outT"].T.astype(np.float32) for r in res.results], axis=0) + b_ho
    new_hidden = np.concatenate(
        [r["nhT"].T.astype(np.float32) for r in res.results], axis=0)
    return output, new_hidden


# revision 47
# speedup vs baseline: 1.0030x; 1.0030x over previous
"""Fused RNN cell on 8 Trainium2 NeuronCores.

Reference computation (fp32):
    combined   = [x, hidden]                      [B=4096, I+H=4096]
    new_hidden = tanh(combined @ W_ih^T + b_ih)   [B, H=2048]
    output     = new_hidden @ W_ho^T + b_ho       [B, O=2048]
    returns (output, new_hidden)

Strategy: data-parallel over the batch — each of the 8 cores processes 512
batch rows with replicated weights; no collectives. All operand layout
transforms (transposes into PE-friendly [K-partition, free] form) happen on
the host so every device DMA is a fat, fully contiguous transfer:

    c   [128, 32, 512]      cL[ki, ko, b]       = combined[b, ko*128+ki]
    w1  [128, 32, 16, 128]  w1L[ki, ko, hc, h]  = W_ih[hc*128+h, ko*128+ki]
    w2  [128, 16, 16, 128]  w2L[hi, ho, oc, o]  = W_ho[oc*128+o, ho*128+hi]
    b1  [128, 16]           b1L[p, hc]          = b_ih[hc*128+p]

All matmul operands are fp16 (full PE rate, 1 col/cycle; fp32 PSUM
accumulation; rms rel err ~5e-4), which halves HBM traffic vs fp32 —
28 MB in + 4 MB out per core vs the ~166 us PE floor, so the kernel is
cleanly compute-bound. (fp8 DoubleRow was measured at 2x PE rate but
e4m3's 3-bit mantissa gives ~5% rms output error vs the 2e-2 gate, and
hi/lo-split schemes cost >= 2 double-rate matmuls = no gain over fp16,
so 16-bit is the fastest dtype that meets accuracy.)

DMA is split across the two HWDGE rings so neither saturates (~186 GB/s
per-ring packet-rate limit) and the first tiles land ASAP: the sync
ring carries only weight loads (w1, w2); the scalar (ACT) ring carries
c loads, b1, nh stores, and out stores. The whole first mm1 group uses
1-ko w1 slices (256 KB) — while the ring pipeline fills, halving the
per-slice transfer keeps delivery ahead of the PE (measured: zero
matmul stalls); the rest uses 0.5 MB 2-ko slices to halve trigger
overhead (~0.65 us queue time per DMA trigger). Dummy matmuls at t=0
warm the PE clock gate (HAM, ~3.4 us busy to reach 2.4 GHz) during the
initial DMA ramp, and a 1-element tanh preloads the ACT table set
(~2.7 us) behind the first c triggers.

mm1 drains: 8 back-to-back tanh ACTs per PSUM group (bias fused), then
the 8 nh store triggers — triggers never sit between ACT ops, so bank
turnaround stays at copy cadence and the next group never stalls. mm2
drains alternate DVE/ACT copies the same way; out-store triggers ride
the scalar ring after the group's copies. mm2 groups are [8, 6, 2] so
only two small stores (on parallel rings) remain after the final
matmul; the final contraction step stops the scalar-ring chunk one
matmul early so its slower drain path gets a head start, and tiny
keep-warm dump-stores keep both DGE rings active into the tail
(measured ~0.3-0.6 us on the exec tail, A/B over drift-interleaved
batches).

Measured: ~183 us HW exec = ~6.5 us framework prologue + ~4 us DMA
fill/clock ramp + ~166 us back-to-back matmuls (PE roofline, zero
stalls) + ~3 us store tail + ~2 us teardown. Rel err ~4.7e-4.
"""

import numpy as np

import concourse.bass as bass
import concourse.mybir as mybir
import concourse.tile as tile
from concourse import bacc, bass_utils

NCORES = 8
B, I, H, O = 4096, 2048, 2048, 2048
BC = B // NCORES          # 512 batch rows per core
K1 = I + H                # mm1 contraction dim, 4096
KO1 = K1 // 128           # 32 k-chunks for mm1
HC = H // 128             # 16 h-chunks
OC = O // 128             # 16 o-chunks
G = 8                     # h-chunks per PSUM group (8 banks)
P = 128
F32 = mybir.dt.float32
F16 = mybir.dt.float16
AF = mybir.ActivationFunctionType
NWARM = 34                # dummy matmuls covering the DMA ramp (~107ns each)


def _build():
    nc = bacc.Bacc("TRN2", target_bir_lowering=False,
                   enable_partition_id=False)

    c = nc.dram_tensor("c", [P, KO1, BC], F16, kind="ExternalInput")
    w1 = nc.dram_tensor("w1", [P, KO1, HC, P], F16, kind="ExternalInput")
    b1 = nc.dram_tensor("b1", [P, HC], F32, kind="ExternalInput")
    w2 = nc.dram_tensor("w2", [P, HC, OC, P], F16, kind="ExternalInput")
    nhT = nc.dram_tensor("nhT", [H, BC], F16, kind="ExternalOutput")
    outT = nc.dram_tensor("outT", [O, BC], F16, kind="ExternalOutput")
    # 128-byte dump targets for the tail keep-warm stores.
    dump_a = nc.dram_tensor("dump_a", [1, 64], F16, kind="Internal")
    dump_b = nc.dram_tensor("dump_b", [1, 64], F16, kind="Internal")

    with tile.TileContext(nc) as tc:
        with tc.tile_pool(name="cpool", bufs=1) as cpool, \
             tc.tile_pool(name="wpool", bufs=10) as wpool, \
             tc.tile_pool(name="nhpool", bufs=1) as nhpool, \
             tc.tile_pool(name="opool", bufs=8) as opool, \
             tc.tile_pool(name="bpool", bufs=1) as bpool, \
             tc.tile_pool(name="ps", bufs=8, space="PSUM") as ps:

            # Warm-up operand first: the sooner the memset lands, the
            # sooner the PE warm-ups can start spinning up the clock.
            warm_sb = bpool.tile([P, P], mybir.dt.bfloat16)
            nc.vector.memset(warm_sb[:], 0.0)

            b1_sb = bpool.tile([P, HC], F32)

            c_sb = cpool.tile([P, KO1, BC], F16)
            nh_sb = nhpool.tile([P, HC, BC], F16)

            # Scalar-ring head: the first c chunks (needed by the first
            # real matmuls; the very first as its own small transfer so
            # matmuls can start before the rest lands) go out before the
            # ACT tanh-table preload blocks the queue for ~2.7 us.
            nc.scalar.dma_start(c_sb[:, 0:1], c[:, 0:1])
            nc.scalar.dma_start(c_sb[:, 1:2], c[:, 1:2])
            nc.scalar.dma_start(c_sb[:, 2:4], c[:, 2:4])
            nc.scalar.dma_start(c_sb[:, 4:6], c[:, 4:6])
            act_warm = bpool.tile([1, 1], F32)
            nc.scalar.activation(act_warm[:], warm_sb[:1, :1], AF.Tanh)
            for ko0 in range(6, KO1, 2):
                nc.scalar.dma_start(c_sb[:, ko0:ko0 + 2], c[:, ko0:ko0 + 2])
            # b_ih rides the scalar ring behind the c stream (it isn't
            # needed until the first group drains, ~60 us in); keeping it
            # off GpSimd SWDGE avoids the expensive dge_drain at teardown.
            nc.scalar.dma_start(b1_sb[:], b1[:])

            # mm1: nh^T[h, b] = tanh(W_ih @ combined^T + b_ih)
            # Two G-sized PSUM groups ping-pong across the 8 banks.
            for g in range(HC // G):
                psums = [ps.tile([P, BC], F32, tag="ps", name=f"ps{i}")
                         for i in range(G)]
                if g == 0:
                    # PE warm-up: HAM holds the PE at 1.2 GHz until ~3.4 us
                    # of busy time. Dummy matmuls (into the last bank this
                    # group will touch; start=True on the real group clears
                    # it) keep the PE active while the first tiles stream
                    # in, so real matmuls run near 2.4 GHz from the start.
                    for _ in range(NWARM):
                        nc.tensor.matmul(
                            psums[G - 1][:, :P], lhsT=warm_sb[:],
                            rhs=warm_sb[:],
                            start=True, stop=True, skip_group_check=True,
                        )
                    # 256 KB 1-ko slices throughout the first group: the
                    # ring pipeline is still filling, and halving the
                    # per-slice transfer keeps delivery ahead of the PE.
                    slices = [(ko, 1, 0, G) for ko in range(KO1)]
                else:
                    slices = [(ko, 2, 0, G) for ko in range(0, KO1, 2)]
                for ko0, kw, i0, ni in slices:
                    w1_sb = wpool.tile([P, 2, G, P], F16, tag="w")
                    nc.sync.dma_start(
                        w1_sb[:, :kw, :ni],
                        w1[:, ko0:ko0 + kw, g * G + i0:g * G + i0 + ni])
                    for kk in range(kw):
                        for i in range(ni):
                            nc.tensor.matmul(
                                psums[i0 + i][:],
                                lhsT=w1_sb[:, kk, i],
                                rhs=c_sb[:, ko0 + kk],
                                start=(ko0 + kk == 0),
                                stop=(ko0 + kk == KO1 - 1),
                            )
                # Back-to-back tanhs first (bank turnaround at ACT copy
                # cadence), store triggers after.
                for i in range(G):
                    hc = g * G + i
                    nc.scalar.activation(
                        nh_sb[:, hc], psums[i][:], AF.Tanh,
                        bias=b1_sb[:, hc:hc + 1],
                    )
                for i in range(G):
                    hc = g * G + i
                    nc.scalar.dma_start(
                        nhT[hc * P:(hc + 1) * P, :], nh_sb[:, hc])

            # mm2: out^T[o, b] = W_ho @ nh^T (+ b_ho on host)
            # Groups of [8, 6, 2] o-chunks: consecutive groups ping-pong
            # through the 8 PSUM banks, and the final drain after the last
            # matmul is just two chunks on two parallel rings.
            for g0, gsz in ((0, 8), (8, 6), (14, 2)):
                psums = [ps.tile([P, BC], F32, tag="ps", name=f"ps{i}")
                         for i in range(gsz)]
                for ho0 in range(0, HC, 2):
                    w2_sb = wpool.tile([P, 2, G, P], F16, tag="w",
                                       name="w2_sb")[:, :, :gsz]
                    nc.sync.dma_start(
                        w2_sb[:], w2[:, ho0:ho0 + 2, g0:g0 + gsz])
                    if gsz == 2 and ho0 == HC - 2:
                        # Keep-warm: tiny dump-stores on both DGE rings,
                        # dependent on the final w2 slice load, so the
                        # engines are freshly active when the two real
                        # tail store triggers land moments later.
                        nc.sync.dma_start(dump_a[:], w2_sb[:1, 0, 0, :64])
                        nc.scalar.dma_start(dump_b[:], w2_sb[:1, 0, 1, :64])
                    for kk in range(2):
                        order = range(gsz)
                        if gsz == 2 and ho0 == HC - 2 and kk == 1:
                            # Final step: stop chunk 15 (the slower
                            # ACT+scalar drain path) one matmul BEFORE
                            # chunk 14, so its copy starts ~216 ns early.
                            order = (1, 0)
                        for i in order:
                            nc.tensor.matmul(
                                psums[i][:],
                                lhsT=w2_sb[:, kk, i],
                                rhs=nh_sb[:, ho0 + kk],
                                start=(ho0 + kk == 0),
                                stop=(ho0 + kk == HC - 1),
                            )
                # Alternate DVE/ACT copies back-to-back, then the store
                # triggers (scalar ring; the sync ring takes the chunk that
                # stops last so the final drain runs on parallel rings).
                o_sbs = []
                for i in range(gsz):
                    o_sb = opool.tile([P, BC], F16, tag="osb")
                    if i % 2:
                        nc.scalar.activation(o_sb[:], psums[i][:], AF.Copy)
                    else:
                        nc.vector.tensor_copy(o_sb[:], psums[i][:])
                    o_sbs.append(o_sb)
                for i in range(gsz):
                    oc = g0 + i
                    eng = nc.sync if (gsz == 2 and i == 0) else nc.scalar
                    eng.dma_start(outT[oc * P:(oc + 1) * P, :], o_sbs[i][:])

    nc.compile()
    return nc


def _shard_inputs(x, hidden, W_ih, b_ih, W_ho, b_ho):
    combined = np.concatenate([x, hidden], axis=1)  # [B, K1]
    w1L = np.ascontiguousarray(
        W_ih.reshape(HC, P, KO1, P).transpose(3, 2, 0, 1).astype(np.float16)
    )  # [ki, ko, hc, h]
    w2L = np.ascontiguousarray(
        W_ho.reshape(OC, P, HC, P).transpose(3, 2, 0, 1).astype(np.float16)
    )  # [hi, ho, oc, o]
    b1L = np.ascontiguousarray(b_ih.reshape(HC, P).T)
    in_maps = []
    for cix in range(NCORES):
        cc = combined[cix * BC:(cix + 1) * BC]  # [BC, K1]
        cL = np.ascontiguousarray(
            cc.reshape(BC, KO1, P).transpose(2, 1, 0).astype(np.float16))
        in_maps.append(
            {"c": cL, "w1": w1L, "b1": b1L, "w2": w2L}
        )
    return in_maps


def _run(in_maps, **kwargs):
    nc = _build()
    return bass_utils.run_bass_kernel_spmd(
        nc, in_maps, core_ids=list(range(NCORES)), **kwargs
    )


def kernel(x, hidden, W_ih, b_ih, W_ho, b_ho):
    x = np.asarray(x, dtype=np.float32)
    hidden = np.asarray(hidden, dtype=np.float32)
    W_ih = np.asarray(W_ih, dtype=np.float32)
    b_ih = np.asarray(b_ih, dtype=np.float32)
    W_ho = np.asarray(W_ho, dtype=np.float32)
    b_ho = np.asarray(b_ho, dtype=np.float32)

    in_maps = _shard_inputs(x, hidden, W_ih, b_ih, W_ho, b_ho)
    res = _run(in_maps)
    output = np.concatenate(
        [r["outT"].T.astype(np.float32) for r in res.results], axis=0) + b_ho
    new_hidden = np.concatenate(
        [r["nhT"].T.astype(np.float32) for r in res.results], axis=0)
    return output, new_hidden
